# revision 37
# baseline (speedup 1.0000x reference)
"""CRF integration (nn_CRFIntegrationModule) Trainium2 kernel — v2.

One image per NeuronCore (B=8 -> 8 cores). Each direction's 32-step scan is
a hardware tensor_tensor_scan (fp32 carry, bf16 i/o) plus a windowed
correction:

    A_inf[n] = (A_inf[n-1] + u[n-1]) * t[n-1]        (one DVE scan op)
    T_32[n]  = prod_{j=1..32} t[n-j]                 (5 doubling multiplies)
    A_32     = A_inf - T_32 * shift(A_inf, 32)       (exact windowed sum)

v2 layout strategy: the host pre-packs BOTH layouts (row-major segments for
the horizontal scans, transposed column-chunks for the vertical scans) as
fp16 DRAM tensors with pads baked in, so the device does no staging
transposes on the input side.  All elementwise work runs in fp16 (2x DVE
tensor_tensor, 4x tensor_scalar; fp32 scan carries); the windowed mask
products run on the otherwise-idle Pool engine; the windowed plog sums use
an exclusive f32 prefix scan + shifted difference; only the V-phase results
are transposed back on-chip (PE + ACT copies), and the final blend reads
the H-phase tiles directly from SBUF, with non-tail blends on Pool.
tolerance budget 2e-2 >> fp16 rounding (measured ~7e-3).
"""
import os
import sys

for _p in ("/opt/trn_rl_repo", "/root/.axon_site/_ro/trn_rl_repo"):
    if os.path.isdir(_p) and _p not in sys.path:
        sys.path.insert(0, _p)
        break

import numpy as np
import ml_dtypes
import concourse.bacc as bacc
import concourse.mybir as mybir
import concourse.tile as tile
from concourse import masks
from concourse.bass_utils import run_bass_kernel_spmd

Alu = mybir.AluOpType
ActF = mybir.ActivationFunctionType
F32 = mybir.dt.float32
F16 = mybir.dt.float16
NF16 = np.float16

B, H, W = 8, 352, 1216
R = 32          # MAXRANGE
CLIP = 5.0      # CLIPVARIANCE
PAD = 32
EMIN = float(np.exp(-CLIP))

# H-phase geometry: row segments (partitions = rows), row-major free axis
RSEGS = [(0, 128), (128, 128), (256, 96)]
FH = W + 2 * PAD + 8                               # 1288 (+8: M[n+33] slack)

# V-phase geometry: transposed layout, 2 chunks x 5 col-segments of <=128 cols
VSEG = H + PAD                                     # 384 per col-seg span
NCS = 5
FV = PAD + NCS * VSEG + 8                          # 1960
VCHUNKS = [(0, 640), (640, 576)]                   # (col0, width)
VLO, VHI = PAD, PAD + (NCS - 1) * VSEG + H         # 32, 1920
HLO, HHI = PAD, PAD + W                            # 32, 1248


def _win_chain_g(nc, dst, t, g1, g2, F, eng=None):
    """dst[n] = prod_{j=1..32} t[n-j] via doubling, on the (idle) Pool
    engine by default.  Exact for 0/1 masks in fp16."""
    gt = (eng or nc.gpsimd).tensor_tensor
    gt(g1[:, 2:F], t[:, 1:F - 1], t[:, 0:F - 2], op=Alu.mult)
    gt(g2[:, 4:F], g1[:, 4:F], g1[:, 2:F - 2], op=Alu.mult)
    gt(g1[:, 8:F], g2[:, 8:F], g2[:, 4:F - 4], op=Alu.mult)
    gt(g2[:, 16:F], g1[:, 16:F], g1[:, 8:F - 8], op=Alu.mult)
    gt(dst[:, 32:F], g2[:, 32:F], g2[:, 16:F - 16], op=Alu.mult)


def _dir_pair(nc, m, p, E0, E1, t_l, t_r, u0, u1, AL, BL, AR, BR,
              TL, TR, M, s1, s2, g1, g2, CP, lo, hi, F, chain_eng=None,
              b_eng=None, bs1=None, bs2=None, seg3d=False):
    """Both directions of one axis on [lo,hi) real region of width-F planes.
    fp16 tiles (mask windows exact 0/1; scans keep an fp32 carry).
    M = windowed mask product on Pool; W32 = windowed plog sum via an
    exclusive f32 prefix scan + shifted difference (exact integers / tiny
    sums, no cancellation issue).  Outputs: AL = awd, BL = aw."""
    v = nc.vector
    if g1 is not None:
        _win_chain_g(nc, M, m, g1, g2, F, eng=chain_eng)

    def ap(X, d=0):
        # segmented 3D view skipping the interior pad strips (V layout)
        if not seg3d:
            return X[:, lo + d:hi + d]
        b0 = max(0, lo + d - (VSEG - H))
        kk = lo + d - b0
        return X[:, b0:b0 + NCS * VSEG].rearrange(
            "p (s c) -> p s c", s=NCS)[:, :, kk:kk + H]

    hiW = hi + R
    v.tensor_tensor_scan(CP[:, 1:hiW], p[:, 0:hiW - 1], p[:, 0:hiW - 1],
                         0.0, op0=Alu.add, op1=Alu.bypass)
    v.scalar_tensor_tensor(TR[:, lo:hiW], CP[:, lo:hiW], 0.0,
                           CP[:, lo - R:hi], op0=Alu.bypass,
                           op1=Alu.subtract)
    # T_L[n] = M[n]*exp(W32[n]);  T_R[n] = M[n+33]*exp(-W32[n+32])
    nc.scalar.activation(s1[:, lo:hi], TR[:, lo:hi], ActF.Exp)
    nc.scalar.activation(s2[:, lo:hi], TR[:, lo + R:hi + R], ActF.Exp,
                         scale=-1.0)
    # scans issue before the M-dependent muls: the in-order DVE queue must
    # not park on the Pool mask-chain while scan inputs are already ready.
    # B scans go first (their inputs E/m are ready before the u-preps).
    v.tensor_tensor_scan(BL[:, lo:hi], E0[:, lo - 1:hi - 1], m[:, lo - 1:hi - 1],
                         0.0, op0=Alu.add, op1=Alu.mult)
    v.tensor_tensor_scan(BR[:, lo:hi][:, ::-1], E1[:, lo + 1:hi + 1][:, ::-1],
                         m[:, lo + 1:hi + 1][:, ::-1], 0.0,
                         op0=Alu.add, op1=Alu.mult)
    v.tensor_tensor_scan(AL[:, lo:hi], u0[:, lo - 1:hi - 1], t_l[:, lo - 1:hi - 1],
                         0.0, op0=Alu.add, op1=Alu.mult)
    v.tensor_tensor_scan(AR[:, lo:hi][:, ::-1], u1[:, lo + 1:hi + 1][:, ::-1],
                         t_r[:, lo + 1:hi + 1][:, ::-1], 0.0,
                         op0=Alu.add, op1=Alu.mult)
    v.tensor_mul(ap(TL), ap(M), ap(s1))
    v.tensor_mul(ap(TR), ap(M, R + 1), ap(s2))
    # corrections: X32 = X - T*shift(X, 32); M_R[n] = M_L[n+33]
    v.tensor_mul(ap(s1), ap(TL), ap(AL, -R))
    v.tensor_sub(ap(AL), ap(AL), ap(s1))
    v.tensor_mul(ap(s2), ap(TR), ap(AR, R))
    v.tensor_sub(ap(AR), ap(AR), ap(s2))
    b = b_eng or v
    if bs1 is None:
        bs1, bs2 = s1, s2
    b.tensor_mul(ap(bs1), ap(M), ap(BL, -R))
    b.tensor_sub(ap(BL), ap(BL), ap(bs1))
    b.tensor_mul(ap(bs2), ap(M, R + 1), ap(BR, R))
    b.tensor_sub(ap(BR), ap(BR), ap(bs2))
    v.tensor_add(ap(AL), ap(AL), ap(AR))
    b.tensor_add(ap(BL), ap(BL), ap(BR))


def _transpose_out(nc, pp, ident, src, stag, c0, cw):
    """src [128, FV] transposed layout -> stag [128, (seg, W)] row-major at
    column offset c0. Full-width col-seg groups merge into one PSUM tile +
    one contiguous ACT copy."""
    ncs = (cw + 127) // 128
    for rp, (r0, hs) in enumerate(RSEGS):
        cs = 0
        while cs < ncs:
            bw = min(128, cw - cs * 128)
            fb = PAD + cs * VSEG + rp * 128
            c = rp * W + c0 + cs * 128
            ng = 0
            while (cs + ng < ncs and ng < 4
                   and min(128, cw - (cs + ng) * 128) == 128):
                ng += 1
            if ng >= 2:
                ps = pp.tile([128, 128 * ng], F16, tag="pt2", bufs=5,
                             name="psg")
                for g in range(ng):
                    nc.tensor.transpose(ps[0:hs, 128 * g:128 * (g + 1)],
                                        src[:, fb + VSEG * g:fb + VSEG * g + hs],
                                        ident[:, :])
                nc.scalar.copy(stag[0:hs, c:c + 128 * ng],
                               ps[0:hs, 0:128 * ng])
                cs += ng
            else:
                ps = pp.tile([128, 128], F16, tag="pt", bufs=3)
                nc.tensor.transpose(ps[0:hs, 0:bw], src[0:bw, fb:fb + hs],
                                    ident[0:bw, 0:bw])
                nc.scalar.copy(stag[0:hs, c:c + bw], ps[0:hs, 0:bw])
                cs += 1


def _v_phase(nc, tc, pp, ident, vcol, rmw, rmwd, qp):
    v = nc.vector
    lo, hi = VLO, VHI
    with tc.tile_pool(name="vp", bufs=1) as vp:
        def t_(tag, bufs=1):
            return vp.tile([128, FV], F16, tag=tag, name=tag, bufs=bufs)

        tu, td, uu, ud = t_("tu"), t_("td"), t_("uu"), t_("ud")
        AR, BR = t_("vAR"), t_("vBR")
        TL, TR = t_("vTL"), t_("vTR")
        s1, s2 = t_("vs1"), t_("vs2")
        bs1, bs2 = t_("vbs1"), t_("vbs2")
        CP = vp.tile([128, FV], F32, tag="vCP", name="vCP")
        nc.gpsimd.memset(CP[:, 0:1], 0.0)
        # one-time edge-strip zeroing (scan/correction shifted reads touch
        # these; SBUF garbage could be NaN and NaN*0 = NaN)
        for t in (tu, td, uu, ud, AR, BR):
            nc.gpsimd.memset(t[:, 0:lo], 0.0)
            nc.gpsimd.memset(t[:, hi:FV], 0.0)
        for t in (tu, td, uu, ud):
            for sg in range(1, NCS):
                nc.gpsimd.memset(t[:, sg * VSEG:sg * VSEG + PAD], 0.0)

        prev = None
        for c, (c0, cw) in enumerate(VCHUNKS):
            # double-buffered tiles: re-fetch per chunk to rotate buffers
            mT, DT = t_("mT", 2), t_("DT", 2)
            e2, e3, pv = t_("e2", 2), t_("e3", 2), t_("pv", 2)
            g1, g2, M = t_("vg1", 2), t_("vg2", 2), t_("vM", 2)
            # V results live in the persistent pool (read by the deferred
            # transpose-outs emitted one chunk later / in the H phase)
            AL = qp.tile([128, FV], F16, tag="vAL", name="vAL", bufs=2)
            BL = qp.tile([128, FV], F16, tag="vBL", name="vBL", bufs=2)
            for i, t in ((0, mT), (2, e2), (3, e3), (4, pv), (1, DT)):
                nc.sync.dma_start(t[:], vcol[i, c])
            # zero the current buffer's edge strips (on DVE: tiny, and
            # keeps the Pool queue free for the mask chains)
            for t in (AL, BL):
                v.memset(t[:, 0:lo], 0.0)
                v.memset(t[:, hi:FV], 0.0)
            # E = max(exp(-var), e^-CLIP)  ==  exp(-min(var, CLIP))
            nc.scalar.activation(e2[:, lo:hi], e2[:, lo:hi], ActF.Exp,
                                 scale=-1.0)
            nc.scalar.activation(e3[:, lo:hi], e3[:, lo:hi], ActF.Exp,
                                 scale=-1.0)
            def a3(X, d=0):
                b0 = max(0, lo + d - (VSEG - H))
                kk = lo + d - b0
                return X[:, b0:b0 + NCS * VSEG].rearrange(
                    "p (s c) -> p s c", s=NCS)[:, :, kk:kk + H]
            v.tensor_scalar_max(a3(e2), a3(e2), EMIN)
            v.tensor_scalar_max(a3(e3), a3(e3), EMIN)
            nc.scalar.activation(s1[:, lo:hi], pv[:, lo:hi], ActF.Exp)
            nc.scalar.activation(s2[:, lo - 1:hi], pv[:, lo - 1:hi],
                                 ActF.Exp, scale=-1.0)
            v.tensor_mul(a3(tu), a3(mT), a3(s1))
            v.tensor_mul(a3(td), a3(mT), a3(s2, -1))
            v.tensor_mul(a3(uu), a3(e2), a3(DT))
            v.tensor_mul(a3(ud), a3(e3), a3(DT))
            if prev is not None:
                # previous chunk's transpose-outs: emitted here so this
                # chunk's exps get the ACT engine first (copies fill the gap)
                pAL, pBL, pc0, pcw = prev
                _transpose_out(nc, pp, ident, pAL, rmwd, pc0, pcw)
                _transpose_out(nc, pp, ident, pBL, rmw, pc0, pcw)

            _dir_pair(nc, mT, pv, e2, e3, tu, td, uu, ud, AL, BL, AR, BR,
                      TL, TR, M, s1, s2, g1, g2, CP, lo, hi, FV,
                      chain_eng=nc.vector if c == 0 else None, seg3d=True)
            prev = (AL, BL, c0, cw)
        return prev


def _h_phase(nc, tc, pp, ident, hrow, dout, rmw, rmwd, lam_t, eps_t, lp,
             vres):
    v = nc.vector
    lo, hi = HLO, HHI
    with tc.tile_pool(name="hp", bufs=1) as hp:
        def t_(tag, bufs=1, w=FH, dt=F16):
            return hp.tile([128, w], dt, tag=tag, name=tag, bufs=bufs)

        tl, tr, u0, u1 = t_("tl"), t_("tr"), t_("u0"), t_("u1")
        AR, BR = t_("AR"), t_("BR")
        TL, TR = t_("TL"), t_("TR")
        s1, s2 = t_("s1"), t_("s2")
        CP = hp.tile([128, FH], F32, tag="hCP", name="hCP")
        nc.gpsimd.memset(CP[:, 0:1], 0.0)
        for t in (tl, tr, u0, u1, AR, BR):
            nc.gpsimd.memset(t[:, 0:lo], 0.0)
            nc.gpsimd.memset(t[:, hi:FH], 0.0)

        # prefetch all three segment masks and run their window chains on
        # Pool upfront (they only need the mask loads)
        mhs, Ms = [], []
        g1 = lp.tile([128, FH], F16, tag="hg1", name="hg1", bufs=2)
        g2 = lp.tile([128, FH], F16, tag="hg2", name="hg2", bufs=2)
        for s in range(3):
            mh = lp.tile([128, FH], F16, tag="mh", name="mh", bufs=3)
            M = lp.tile([128, FH], F16, tag="hM", name="hM", bufs=3)
            nc.sync.dma_start(mh[:], hrow[0, s])
            if s > 0:
                g1 = lp.tile([128, FH], F16, tag="hg1", name="hg1", bufs=2)
                g2 = lp.tile([128, FH], F16, tag="hg2", name="hg2", bufs=2)
            _win_chain_g(nc, M, mh, g1, g2, FH)
            mhs.append(mh)
            Ms.append(M)

        def emit_blend(e, s, hs, AL_s, BL_s, Do_s, mlam_s, pieces,
                       te=None):
            """Blend + store for segment s on engine e (v or gpsimd); the
            sel comparison stays on DVE (Pool lacks TensorScalar).  te
            overrides the engine for the post-reciprocal tail ops."""
            te = te or e
            tww = t_("tww", 2, W)
            twdw = t_("twdw", 2, W)
            sel = t_("sel", 2, W)
            blo = t_("blo", 2, W)
            rcl = t_("rcl", 2, W, F32)
            rcb = t_("rcb", 2, W, F32)
            for c0, cw in pieces:
                hc = slice(lo + c0, lo + c0 + cw)
                rs_ = slice(s * W + c0, s * W + c0 + cw)
                bs = slice(c0, c0 + cw)
                e.tensor_add(tww[:, bs], BL_s[:, hc], rmw[:, rs_])
                e.tensor_add(twdw[:, bs], AL_s[:, hc], rmwd[:, rs_])
                v.tensor_scalar(sel[:, bs], tww[:, bs], 0.0, None,
                                op0=Alu.is_gt)
                te.tensor_mul(sel[:, bs], sel[:, bs], mlam_s[:, hc])
                # 1/tw via exp(-ln(tw + 1e-6)); ln kept in f32 for accuracy
                nc.scalar.activation(rcl[:, bs], tww[:, bs], ActF.Ln,
                                     bias=eps_t[:, 0:1])
                nc.scalar.activation(rcb[:, bs], rcl[:, bs], ActF.Exp,
                                     scale=-1.0)
                te.tensor_mul(blo[:, bs], twdw[:, bs], rcb[:, bs])
                te.tensor_sub(blo[:, bs], blo[:, bs], Do_s[:, hc])
                te.tensor_mul(blo[:, bs], blo[:, bs], sel[:, bs])
                te.tensor_add(blo[:, bs], blo[:, bs], Do_s[:, hc])
                nc.sync.dma_start(dout[s, 0:hs, c0:c0 + cw],
                                  blo[0:hs, bs])

        pend = None
        for s, (r0, hs) in enumerate(RSEGS):
            mh, M = mhs[s], Ms[s]
            # double-buffered tiles: re-fetch per segment to rotate buffers
            Do = lp.tile([128, FH], F16, tag="Do", name="Do", bufs=3)
            Dc = lp.tile([128, FH], F16, tag="Dc", name="Dc", bufs=2)
            e0 = lp.tile([128, FH], F16, tag="e0", name="e0", bufs=2)
            e1 = lp.tile([128, FH], F16, tag="e1", name="e1", bufs=2)
            ph = lp.tile([128, FH], F16, tag="ph", name="ph", bufs=2)
            AL = t_("AL", 2)
            BL = t_("BL", 2)
            mlam = t_("mlam", 2)
            for i, t in ((3, e0), (4, e1), (5, ph), (2, Dc), (1, Do)):
                nc.sync.dma_start(t[:], hrow[i, s])
            for t in (AL, BL):
                nc.gpsimd.memset(t[:, 0:lo], 0.0)
                nc.gpsimd.memset(t[:, hi:FH], 0.0)
            nc.scalar.activation(e0[:, lo:hi], e0[:, lo:hi], ActF.Exp,
                                 scale=-1.0)
            nc.scalar.activation(e1[:, lo:hi], e1[:, lo:hi], ActF.Exp,
                                 scale=-1.0)
            v.tensor_scalar_max(e0[:, lo:hi], e0[:, lo:hi], EMIN)
            v.tensor_scalar_max(e1[:, lo:hi], e1[:, lo:hi], EMIN)
            nc.scalar.activation(s1[:, lo:hi], ph[:, lo:hi], ActF.Exp)
            nc.scalar.activation(s2[:, lo - 1:hi], ph[:, lo - 1:hi],
                                 ActF.Exp, scale=-1.0)
            v.tensor_mul(tl[:, lo:hi], mh[:, lo:hi], s1[:, lo:hi])
            v.tensor_mul(tr[:, lo:hi], mh[:, lo:hi], s2[:, lo - 1:hi - 1])
            v.tensor_mul(u0[:, lo:hi], e0[:, lo:hi], Dc[:, lo:hi])
            v.tensor_mul(u1[:, lo:hi], e1[:, lo:hi], Dc[:, lo:hi])
            nc.scalar.activation(mlam[:, lo:hi], mh[:, lo:hi], ActF.Copy,
                                 scale=lam_t[:, 0:1])

            _dir_pair(nc, mh, ph, e0, e1, tl, tr, u0, u1, AL, BL, AR, BR,
                      TL, TR, M, s1, s2, None, None, CP, lo, hi, FH)

            if s == 0 and vres is not None:
                # deferred transpose-out of the last V chunk: emitted after
                # seg0's ACT ops so the H exps aren't queued behind copies
                vAL, vBL, vc0, vcw = vres
                _transpose_out(nc, pp, ident, vAL, rmwd, vc0, vcw)
                _transpose_out(nc, pp, ident, vBL, rmw, vc0, vcw)

            # blends: non-tail segments run on Pool, deferred one segment so
            # their ACT/DVE helper ops never head-of-line-block the next
            # segment's prep; the last segment runs on DVE (short tail),
            # split into half-chunk pieces so stores overlap the blend.
            last = (s == len(RSEGS) - 1)
            if pend is not None:
                # in the last segment the pended blend's tail runs on DVE:
                # DVE is idle at kernel end and 2.5x faster than Pool there
                emit_blend(nc.gpsimd, *pend,
                           [(c0, cw) for c0, cw in VCHUNKS],
                           te=v if last else None)
            if last:
                emit_blend(v, s, hs, AL, BL, Do, mlam, list(VCHUNKS))
            else:
                pend = (s, hs, AL, BL, Do, mlam)


def build_program():
    nc = bacc.Bacc("TRN2", target_bir_lowering=False, debug=False)

    hrow = nc.dram_tensor("hrow", [6, 3, 128, FH], F16,
                          kind="ExternalInput").ap()
    vcol = nc.dram_tensor("vcol", [5, 2, 128, FV], F16,
                          kind="ExternalInput").ap()
    lam = nc.dram_tensor("lam", [1], F32, kind="ExternalInput").ap()
    dout = nc.dram_tensor("dout", [3, 128, W], F16,
                          kind="ExternalOutput").ap()

    # Pin Exp/Ln to the one ACT table set containing both, so the
    # activation-table loader emits a single LoadActFuncSet instead of
    # ping-ponging between exp-only and ln-only sets (1.3us per reload).
    from concourse.hw_specs import get_activation_tables
    _tabs = get_activation_tables(nc.m.arch)
    for _name, _s in _tabs.items():
        if _name != "natural_log_exp_and_others":
            _s.discard(ActF.Exp)
            _s.discard(ActF.Ln)

    with tile.TileContext(nc, pool_alloc_mode="queue") as tc:
        with tc.tile_pool(name="const", bufs=1) as cp, \
             tc.tile_pool(name="psum", bufs=8, space="PSUM") as pp, \
             tc.tile_pool(name="persist", bufs=1) as qp:
            ident = cp.tile([128, 128], F16, tag="ident")
            masks.make_identity(nc, ident[:])
            lam_t = cp.tile([128, 1], F32, tag="lam")
            nc.gpsimd.dma_start(lam_t[:, 0:1], lam.partition_broadcast(128))
            eps_t = cp.tile([128, 1], F32, tag="eps")
            nc.gpsimd.memset(eps_t[:], 1e-6)
            rmw = qp.tile([128, 3 * W], F16, tag="rmw")
            rmwd = qp.tile([128, 3 * W], F16, tag="rmwd")

            vres = _v_phase(nc, tc, pp, ident, vcol, rmw, rmwd, qp)
            _h_phase(nc, tc, pp, ident, hrow, dout, rmw, rmwd, lam_t,
                     eps_t, qp, vres)
    nc.finalize()
    return nc


def _pack_inputs(pred_log, maskf, variance, dorig, dcur):
    """Host-side layout prep: row-major segmented planes for the H phase and
    transposed column-chunk planes for the V phase, pads zeroed, bf16."""
    nb = maskf.shape[0]
    planes = np.stack([maskf, dorig, dcur,
                       variance[:, 0], variance[:, 1], pred_log[:, 0]], 1)
    pb = planes.astype(NF16)
    hrow = np.zeros((nb, 6, 3, 128, FH), NF16)
    for s, (r0, hs) in enumerate(RSEGS):
        hrow[:, :, s, 0:hs, PAD:PAD + W] = pb[:, :, r0:r0 + hs, :]
    vplanes = np.stack([maskf, dcur, variance[:, 2], variance[:, 3],
                        pred_log[:, 1]], 1)
    vT = np.ascontiguousarray(vplanes.transpose(0, 1, 3, 2)).astype(NF16)
    vcol = np.zeros((nb, 5, 2, 128, FV), NF16)
    for c, (c0, cw) in enumerate(VCHUNKS):
        for s in range(NCS):
            bw = min(128, cw - s * 128)
            w0 = c0 + s * 128
            vcol[:, :, c, 0:bw, PAD + s * VSEG:PAD + s * VSEG + H] = \
                vT[:, :, w0:w0 + bw, :]
    return hrow, vcol


def _unpack(dout):
    """dout [3, 128, W] bf16 -> [H, W] f32."""
    return np.concatenate(
        [np.asarray(dout[s][0:hs], np.float32)
         for s, (r0, hs) in enumerate(RSEGS)], axis=0)


_NC = None


def _get_nc():
    global _NC
    if _NC is None:
        _NC = build_program()
    return _NC


def kernel(pred_log, mask, variance, depthin, lam, times):
    pred_log = np.asarray(pred_log, np.float32)
    mask = np.asarray(mask, np.int32)
    variance = np.asarray(variance, np.float32)
    depthin = np.asarray(depthin, np.float32)
    lam = np.asarray(lam, np.float32).reshape(1)
    t = int(np.asarray(times))

    if t <= 0:
        return depthin.copy()
    nc = _get_nc()
    maskf = mask[:, 0].astype(np.float32)
    dorig = depthin[:, 0]
    dcur = dorig
    for _ in range(t):
        hrow, vcol = _pack_inputs(pred_log, maskf, variance, dorig, dcur)
        in_maps = [{"hrow": hrow[b], "vcol": vcol[b], "lam": lam}
                   for b in range(B)]
        res = run_bass_kernel_spmd(nc, in_maps, list(range(B)))
        dcur = np.stack([_unpack(res.results[i]["dout"]) for i in range(B)])
    return dcur[:, None].astype(np.float32)


# revision 38
# speedup vs baseline: 1.0136x; 1.0136x over previous
"""CRF integration (nn_CRFIntegrationModule) Trainium2 kernel — v2.

One image per NeuronCore (B=8 -> 8 cores). Each direction's 32-step scan is
a hardware tensor_tensor_scan (fp32 carry, bf16 i/o) plus a windowed
correction:

    A_inf[n] = (A_inf[n-1] + u[n-1]) * t[n-1]        (one DVE scan op)
    T_32[n]  = prod_{j=1..32} t[n-j]                 (5 doubling multiplies)
    A_32     = A_inf - T_32 * shift(A_inf, 32)       (exact windowed sum)

v2 layout strategy: the host pre-packs BOTH layouts (row-major segments for
the horizontal scans, transposed column-chunks for the vertical scans) as
fp16 DRAM tensors with pads baked in, so the device does no staging
transposes on the input side.  All elementwise work runs in fp16 (2x DVE
tensor_tensor, 4x tensor_scalar; fp32 scan carries); the windowed mask
products run on the otherwise-idle Pool engine; the windowed plog sums use
an exclusive f32 prefix scan + shifted difference; only the V-phase results
are transposed back on-chip (PE + ACT copies), and the final blend reads
the H-phase tiles directly from SBUF, with non-tail blends on Pool.
tolerance budget 2e-2 >> fp16 rounding (measured ~7e-3).
"""
import os
import sys

for _p in ("/opt/trn_rl_repo", "/root/.axon_site/_ro/trn_rl_repo"):
    if os.path.isdir(_p) and _p not in sys.path:
        sys.path.insert(0, _p)
        break

import numpy as np
import ml_dtypes
import concourse.bacc as bacc
import concourse.mybir as mybir
import concourse.tile as tile
from concourse import masks
from concourse.bass_utils import run_bass_kernel_spmd

Alu = mybir.AluOpType
ActF = mybir.ActivationFunctionType
F32 = mybir.dt.float32
F16 = mybir.dt.float16
NF16 = np.float16

B, H, W = 8, 352, 1216
R = 32          # MAXRANGE
CLIP = 5.0      # CLIPVARIANCE
PAD = 32
EMIN = float(np.exp(-CLIP))

# H-phase geometry: row segments (partitions = rows), row-major free axis
RSEGS = [(0, 128), (128, 128), (256, 96)]
FH = W + 2 * PAD + 8                               # 1288 (+8: M[n+33] slack)

# V-phase geometry: transposed layout, 2 chunks x 5 col-segments of <=128 cols
VSEG = H + PAD                                     # 384 per col-seg span
NCS = 5
FV = PAD + NCS * VSEG + 8                          # 1960
VCHUNKS = [(0, 640), (640, 576)]                   # (col0, width)
VLO, VHI = PAD, PAD + (NCS - 1) * VSEG + H         # 32, 1920
HLO, HHI = PAD, PAD + W                            # 32, 1248


def _win_chain_g(nc, dst, t, g1, g2, F, eng=None):
    """dst[n] = prod_{j=1..32} t[n-j] via doubling, on the (idle) Pool
    engine by default.  Exact for 0/1 masks in fp16."""
    gt = (eng or nc.gpsimd).tensor_tensor
    gt(g1[:, 2:F], t[:, 1:F - 1], t[:, 0:F - 2], op=Alu.mult)
    gt(g2[:, 4:F], g1[:, 4:F], g1[:, 2:F - 2], op=Alu.mult)
    gt(g1[:, 8:F], g2[:, 8:F], g2[:, 4:F - 4], op=Alu.mult)
    gt(g2[:, 16:F], g1[:, 16:F], g1[:, 8:F - 8], op=Alu.mult)
    gt(dst[:, 32:F], g2[:, 32:F], g2[:, 16:F - 16], op=Alu.mult)


def _dir_pair(nc, m, p, E0, E1, t_l, t_r, u0, u1, AL, BL, AR, BR,
              TL, TR, M, s1, s2, g1, g2, CP, lo, hi, F, chain_eng=None,
              b_eng=None, bs1=None, bs2=None, seg3d=False):
    """Both directions of one axis on [lo,hi) real region of width-F planes.
    fp16 tiles (mask windows exact 0/1; scans keep an fp32 carry).
    M = windowed mask product on Pool; W32 = windowed plog sum via an
    exclusive f32 prefix scan + shifted difference (exact integers / tiny
    sums, no cancellation issue).  Outputs: AL = awd, BL = aw."""
    v = nc.vector
    if g1 is not None:
        _win_chain_g(nc, M, m, g1, g2, F, eng=chain_eng)

    def ap(X, d=0):
        # segmented 3D view skipping the interior pad strips (V layout)
        if not seg3d:
            return X[:, lo + d:hi + d]
        b0 = max(0, lo + d - (VSEG - H))
        kk = lo + d - b0
        return X[:, b0:b0 + NCS * VSEG].rearrange(
            "p (s c) -> p s c", s=NCS)[:, :, kk:kk + H]

    hiW = hi + R
    v.tensor_tensor_scan(CP[:, 1:hiW], p[:, 0:hiW - 1], p[:, 0:hiW - 1],
                         0.0, op0=Alu.add, op1=Alu.bypass)
    v.scalar_tensor_tensor(TR[:, lo:hiW], CP[:, lo:hiW], 0.0,
                           CP[:, lo - R:hi], op0=Alu.bypass,
                           op1=Alu.subtract)
    # T_L[n] = M[n]*exp(W32[n]);  T_R[n] = M[n+33]*exp(-W32[n+32])
    nc.scalar.activation(s1[:, lo:hi], TR[:, lo:hi], ActF.Exp)
    nc.scalar.activation(s2[:, lo:hi], TR[:, lo + R:hi + R], ActF.Exp,
                         scale=-1.0)
    # scans issue before the M-dependent muls: the in-order DVE queue must
    # not park on the Pool mask-chain while scan inputs are already ready.
    # B scans go first (their inputs E/m are ready before the u-preps).
    v.tensor_tensor_scan(BL[:, lo:hi], E0[:, lo - 1:hi - 1], m[:, lo - 1:hi - 1],
                         0.0, op0=Alu.add, op1=Alu.mult)
    v.tensor_tensor_scan(BR[:, lo:hi][:, ::-1], E1[:, lo + 1:hi + 1][:, ::-1],
                         m[:, lo + 1:hi + 1][:, ::-1], 0.0,
                         op0=Alu.add, op1=Alu.mult)
    v.tensor_tensor_scan(AL[:, lo:hi], u0[:, lo - 1:hi - 1], t_l[:, lo - 1:hi - 1],
                         0.0, op0=Alu.add, op1=Alu.mult)
    v.tensor_tensor_scan(AR[:, lo:hi][:, ::-1], u1[:, lo + 1:hi + 1][:, ::-1],
                         t_r[:, lo + 1:hi + 1][:, ::-1], 0.0,
                         op0=Alu.add, op1=Alu.mult)
    v.tensor_mul(ap(TL), ap(M), ap(s1))
    v.tensor_mul(ap(TR), ap(M, R + 1), ap(s2))
    # corrections: X32 = X - T*shift(X, 32); M_R[n] = M_L[n+33]
    v.tensor_mul(ap(s1), ap(TL), ap(AL, -R))
    v.tensor_sub(ap(AL), ap(AL), ap(s1))
    v.tensor_mul(ap(s2), ap(TR), ap(AR, R))
    v.tensor_sub(ap(AR), ap(AR), ap(s2))
    b = b_eng or v
    if bs1 is None:
        bs1, bs2 = s1, s2
    b.tensor_mul(ap(bs1), ap(M), ap(BL, -R))
    b.tensor_sub(ap(BL), ap(BL), ap(bs1))
    b.tensor_mul(ap(bs2), ap(M, R + 1), ap(BR, R))
    b.tensor_sub(ap(BR), ap(BR), ap(bs2))
    v.tensor_add(ap(AL), ap(AL), ap(AR))
    b.tensor_add(ap(BL), ap(BL), ap(BR))


def _transpose_out(nc, pp, ident, src, stag, c0, cw):
    """src [128, FV] transposed layout -> stag [128, (seg, W)] row-major at
    column offset c0. Full-width col-seg groups merge into one PSUM tile +
    one contiguous ACT copy."""
    ncs = (cw + 127) // 128
    for rp, (r0, hs) in enumerate(RSEGS):
        cs = 0
        while cs < ncs:
            bw = min(128, cw - cs * 128)
            fb = PAD + cs * VSEG + rp * 128
            c = rp * W + c0 + cs * 128
            ng = 0
            while (cs + ng < ncs and ng < 4
                   and min(128, cw - (cs + ng) * 128) == 128):
                ng += 1
            if ng >= 2:
                ps = pp.tile([128, 128 * ng], F16, tag="pt2", bufs=5,
                             name="psg")
                for g in range(ng):
                    nc.tensor.transpose(ps[0:hs, 128 * g:128 * (g + 1)],
                                        src[:, fb + VSEG * g:fb + VSEG * g + hs],
                                        ident[:, :])
                nc.scalar.copy(stag[0:hs, c:c + 128 * ng],
                               ps[0:hs, 0:128 * ng])
                cs += ng
            else:
                ps = pp.tile([128, 128], F16, tag="pt", bufs=3)
                nc.tensor.transpose(ps[0:hs, 0:bw], src[0:bw, fb:fb + hs],
                                    ident[0:bw, 0:bw])
                nc.scalar.copy(stag[0:hs, c:c + bw], ps[0:hs, 0:bw])
                cs += 1


def _v_phase(nc, tc, pp, ident, vcol, rmw, rmwd, qp):
    v = nc.vector
    lo, hi = VLO, VHI
    with tc.tile_pool(name="vp", bufs=1) as vp:
        def t_(tag, bufs=1):
            return vp.tile([128, FV], F16, tag=tag, name=tag, bufs=bufs)

        tu, td, uu, ud = t_("tu"), t_("td"), t_("uu"), t_("ud")
        AR, BR = t_("vAR"), t_("vBR")
        TL, TR = t_("vTL"), t_("vTR")
        s1, s2 = t_("vs1"), t_("vs2")
        bs1, bs2 = t_("vbs1"), t_("vbs2")
        CP = vp.tile([128, FV], F32, tag="vCP", name="vCP")
        nc.gpsimd.memset(CP[:, 0:1], 0.0)
        # one-time edge-strip zeroing (scan/correction shifted reads touch
        # these; SBUF garbage could be NaN and NaN*0 = NaN)
        for t in (tu, td, uu, ud, AR, BR):
            nc.gpsimd.memset(t[:, 0:lo], 0.0)
            nc.gpsimd.memset(t[:, hi:FV], 0.0)
        for t in (tu, td, uu, ud):
            for sg in range(1, NCS):
                nc.gpsimd.memset(t[:, sg * VSEG:sg * VSEG + PAD], 0.0)

        prev = None
        for c, (c0, cw) in enumerate(VCHUNKS):
            # double-buffered tiles: re-fetch per chunk to rotate buffers
            mT, DT = t_("mT", 2), t_("DT", 2)
            e2, e3, pv = t_("e2", 2), t_("e3", 2), t_("pv", 2)
            g1, g2, M = t_("vg1", 2), t_("vg2", 2), t_("vM", 2)
            # V results live in the persistent pool (read by the deferred
            # transpose-outs emitted one chunk later / in the H phase)
            AL = qp.tile([128, FV], F16, tag="vAL", name="vAL", bufs=2)
            BL = qp.tile([128, FV], F16, tag="vBL", name="vBL", bufs=2)
            for i, t in ((0, mT), (2, e2), (3, e3), (4, pv), (1, DT)):
                nc.sync.dma_start(t[:], vcol[i, c])
            # zero the current buffer's edge strips (on DVE: tiny, and
            # keeps the Pool queue free for the mask chains)
            for t in (AL, BL):
                v.memset(t[:, 0:lo], 0.0)
                v.memset(t[:, hi:FV], 0.0)
            # E = max(exp(-var), e^-CLIP)  ==  exp(-min(var, CLIP))
            nc.scalar.activation(e2[:, lo:hi], e2[:, lo:hi], ActF.Exp,
                                 scale=-1.0)
            nc.scalar.activation(e3[:, lo:hi], e3[:, lo:hi], ActF.Exp,
                                 scale=-1.0)
            def a3(X, d=0):
                b0 = max(0, lo + d - (VSEG - H))
                kk = lo + d - b0
                return X[:, b0:b0 + NCS * VSEG].rearrange(
                    "p (s c) -> p s c", s=NCS)[:, :, kk:kk + H]
            v.tensor_scalar_max(a3(e2), a3(e2), EMIN)
            v.tensor_scalar_max(a3(e3), a3(e3), EMIN)
            nc.scalar.activation(s1[:, lo:hi], pv[:, lo:hi], ActF.Exp)
            nc.scalar.activation(s2[:, lo - 1:hi], pv[:, lo - 1:hi],
                                 ActF.Exp, scale=-1.0)
            v.tensor_mul(a3(tu), a3(mT), a3(s1))
            v.tensor_mul(a3(td), a3(mT), a3(s2, -1))
            v.tensor_mul(a3(uu), a3(e2), a3(DT))
            v.tensor_mul(a3(ud), a3(e3), a3(DT))
            if prev is not None:
                # previous chunk's transpose-outs: emitted here so this
                # chunk's exps get the ACT engine first (copies fill the gap)
                pAL, pBL, pc0, pcw = prev
                _transpose_out(nc, pp, ident, pAL, rmwd, pc0, pcw)
                _transpose_out(nc, pp, ident, pBL, rmw, pc0, pcw)

            _dir_pair(nc, mT, pv, e2, e3, tu, td, uu, ud, AL, BL, AR, BR,
                      TL, TR, M, s1, s2, g1, g2, CP, lo, hi, FV,
                      chain_eng=nc.vector if c == 0 else None, seg3d=True)
            prev = (AL, BL, c0, cw)
        return prev


def _h_phase(nc, tc, pp, ident, hrow, dout, rmw, rmwd, lam_t, eps_t, lp,
             vres):
    v = nc.vector
    lo, hi = HLO, HHI
    with tc.tile_pool(name="hp", bufs=1) as hp:
        def t_(tag, bufs=1, w=FH, dt=F16):
            return hp.tile([128, w], dt, tag=tag, name=tag, bufs=bufs)

        tl, tr, u0, u1 = t_("tl"), t_("tr"), t_("u0"), t_("u1")
        AR, BR = t_("AR"), t_("BR")
        TL, TR = t_("TL"), t_("TR")
        s1, s2 = t_("s1"), t_("s2")
        CP = hp.tile([128, FH], F32, tag="hCP", name="hCP")
        nc.gpsimd.memset(CP[:, 0:1], 0.0)
        for t in (tl, tr, u0, u1, AR, BR):
            nc.gpsimd.memset(t[:, 0:lo], 0.0)
            nc.gpsimd.memset(t[:, hi:FH], 0.0)

        # prefetch all three segment masks and run their window chains on
        # Pool upfront (they only need the mask loads)
        mhs, Ms = [], []
        g1 = lp.tile([128, FH], F16, tag="hg1", name="hg1", bufs=2)
        g2 = lp.tile([128, FH], F16, tag="hg2", name="hg2", bufs=2)
        for s in range(3):
            mh = lp.tile([128, FH], F16, tag="mh", name="mh", bufs=3)
            M = lp.tile([128, FH], F16, tag="hM", name="hM", bufs=3)
            nc.sync.dma_start(mh[:], hrow[0, s])
            if s > 0:
                g1 = lp.tile([128, FH], F16, tag="hg1", name="hg1", bufs=2)
                g2 = lp.tile([128, FH], F16, tag="hg2", name="hg2", bufs=2)
            _win_chain_g(nc, M, mh, g1, g2, FH)
            mhs.append(mh)
            Ms.append(M)

        def emit_blend(e, s, hs, AL_s, BL_s, Do_s, mlam_s, pieces,
                       te=None):
            """Blend + store for segment s on engine e (v or gpsimd); the
            sel comparison stays on DVE (Pool lacks TensorScalar).  te
            overrides the engine for the post-reciprocal tail ops."""
            te = te or e
            tww = t_("tww", 2, W)
            twdw = t_("twdw", 2, W)
            sel = t_("sel", 2, W)
            blo = t_("blo", 2, W)
            rcl = t_("rcl", 2, W, F32)
            rcb = t_("rcb", 2, W, F32)
            for c0, cw in pieces:
                hc = slice(lo + c0, lo + c0 + cw)
                rs_ = slice(s * W + c0, s * W + c0 + cw)
                bs = slice(c0, c0 + cw)
                e.tensor_add(tww[:, bs], BL_s[:, hc], rmw[:, rs_])
                e.tensor_add(twdw[:, bs], AL_s[:, hc], rmwd[:, rs_])
                v.tensor_scalar(sel[:, bs], tww[:, bs], 0.0, None,
                                op0=Alu.is_gt)
                te.tensor_mul(sel[:, bs], sel[:, bs], mlam_s[:, hc])
                # 1/tw via exp(-ln(tw + 1e-6)); ln kept in f32 for accuracy
                nc.scalar.activation(rcl[:, bs], tww[:, bs], ActF.Ln,
                                     bias=eps_t[:, 0:1])
                nc.scalar.activation(rcb[:, bs], rcl[:, bs], ActF.Exp,
                                     scale=-1.0)
                te.tensor_mul(blo[:, bs], twdw[:, bs], rcb[:, bs])
                te.tensor_sub(blo[:, bs], blo[:, bs], Do_s[:, hc])
                te.tensor_mul(blo[:, bs], blo[:, bs], sel[:, bs])
                te.tensor_add(blo[:, bs], blo[:, bs], Do_s[:, hc])
                nc.sync.dma_start(dout[s, 0:hs, c0:c0 + cw],
                                  blo[0:hs, bs])

        pend = None
        for s, (r0, hs) in enumerate(RSEGS):
            mh, M = mhs[s], Ms[s]
            # double-buffered tiles: re-fetch per segment to rotate buffers
            Do = lp.tile([128, FH], F16, tag="Do", name="Do", bufs=3)
            Dc = lp.tile([128, FH], F16, tag="Dc", name="Dc", bufs=2)
            e0 = lp.tile([128, FH], F16, tag="e0", name="e0", bufs=2)
            e1 = lp.tile([128, FH], F16, tag="e1", name="e1", bufs=2)
            ph = lp.tile([128, FH], F16, tag="ph", name="ph", bufs=2)
            AL = t_("AL", 2)
            BL = t_("BL", 2)
            mlam = t_("mlam", 2)
            for i, t in ((3, e0), (4, e1), (5, ph), (2, Dc), (1, Do)):
                nc.sync.dma_start(t[:], hrow[i, s])
            for t in (AL, BL):
                nc.gpsimd.memset(t[:, 0:lo], 0.0)
                nc.gpsimd.memset(t[:, hi:FH], 0.0)
            nc.scalar.activation(e0[:, lo:hi], e0[:, lo:hi], ActF.Exp,
                                 scale=-1.0)
            nc.scalar.activation(e1[:, lo:hi], e1[:, lo:hi], ActF.Exp,
                                 scale=-1.0)
            v.tensor_scalar_max(e0[:, lo:hi], e0[:, lo:hi], EMIN)
            v.tensor_scalar_max(e1[:, lo:hi], e1[:, lo:hi], EMIN)
            nc.scalar.activation(s1[:, lo:hi], ph[:, lo:hi], ActF.Exp)
            nc.scalar.activation(s2[:, lo - 1:hi], ph[:, lo - 1:hi],
                                 ActF.Exp, scale=-1.0)
            v.tensor_mul(tl[:, lo:hi], mh[:, lo:hi], s1[:, lo:hi])
            v.tensor_mul(tr[:, lo:hi], mh[:, lo:hi], s2[:, lo - 1:hi - 1])
            v.tensor_mul(u0[:, lo:hi], e0[:, lo:hi], Dc[:, lo:hi])
            v.tensor_mul(u1[:, lo:hi], e1[:, lo:hi], Dc[:, lo:hi])
            nc.scalar.activation(mlam[:, lo:hi], mh[:, lo:hi], ActF.Copy,
                                 scale=lam_t[:, 0:1])

            _dir_pair(nc, mh, ph, e0, e1, tl, tr, u0, u1, AL, BL, AR, BR,
                      TL, TR, M, s1, s2, None, None, CP, lo, hi, FH)

            if s == 0 and vres is not None:
                # deferred transpose-out of the last V chunk: emitted after
                # seg0's ACT ops so the H exps aren't queued behind copies
                vAL, vBL, vc0, vcw = vres
                _transpose_out(nc, pp, ident, vAL, rmwd, vc0, vcw)
                _transpose_out(nc, pp, ident, vBL, rmw, vc0, vcw)

            # blends: non-tail segments run on Pool, deferred one segment so
            # their ACT/DVE helper ops never head-of-line-block the next
            # segment's prep; the last segment runs on DVE (short tail),
            # split into half-chunk pieces so stores overlap the blend.
            last = (s == len(RSEGS) - 1)
            if pend is not None:
                emit_blend(nc.gpsimd, *pend,
                           [(c0, cw) for c0, cw in VCHUNKS])
            if last:
                emit_blend(v, s, hs, AL, BL, Do, mlam, list(VCHUNKS))
            else:
                pend = (s, hs, AL, BL, Do, mlam)


def build_program():
    nc = bacc.Bacc("TRN2", target_bir_lowering=False, debug=False)

    hrow = nc.dram_tensor("hrow", [6, 3, 128, FH], F16,
                          kind="ExternalInput").ap()
    vcol = nc.dram_tensor("vcol", [5, 2, 128, FV], F16,
                          kind="ExternalInput").ap()
    lam = nc.dram_tensor("lam", [1], F32, kind="ExternalInput").ap()
    dout = nc.dram_tensor("dout", [3, 128, W], F16,
                          kind="ExternalOutput").ap()

    # Pin Exp/Ln to the one ACT table set containing both, so the
    # activation-table loader emits a single LoadActFuncSet instead of
    # ping-ponging between exp-only and ln-only sets (1.3us per reload).
    from concourse.hw_specs import get_activation_tables
    _tabs = get_activation_tables(nc.m.arch)
    for _name, _s in _tabs.items():
        if _name != "natural_log_exp_and_others":
            _s.discard(ActF.Exp)
            _s.discard(ActF.Ln)

    with tile.TileContext(nc, pool_alloc_mode="queue") as tc:
        with tc.tile_pool(name="const", bufs=1) as cp, \
             tc.tile_pool(name="psum", bufs=8, space="PSUM") as pp, \
             tc.tile_pool(name="persist", bufs=1) as qp:
            ident = cp.tile([128, 128], F16, tag="ident")
            masks.make_identity(nc, ident[:])
            lam_t = cp.tile([128, 1], F32, tag="lam")
            nc.gpsimd.dma_start(lam_t[:, 0:1], lam.partition_broadcast(128))
            eps_t = cp.tile([128, 1], F32, tag="eps")
            nc.gpsimd.memset(eps_t[:], 1e-6)
            rmw = qp.tile([128, 3 * W], F16, tag="rmw")
            rmwd = qp.tile([128, 3 * W], F16, tag="rmwd")

            vres = _v_phase(nc, tc, pp, ident, vcol, rmw, rmwd, qp)
            _h_phase(nc, tc, pp, ident, hrow, dout, rmw, rmwd, lam_t,
                     eps_t, qp, vres)
    nc.finalize()
    return nc


def _pack_inputs(pred_log, maskf, variance, dorig, dcur):
    """Host-side layout prep: row-major segmented planes for the H phase and
    transposed column-chunk planes for the V phase, pads zeroed, bf16."""
    nb = maskf.shape[0]
    planes = np.stack([maskf, dorig, dcur,
                       variance[:, 0], variance[:, 1], pred_log[:, 0]], 1)
    pb = planes.astype(NF16)
    hrow = np.zeros((nb, 6, 3, 128, FH), NF16)
    for s, (r0, hs) in enumerate(RSEGS):
        hrow[:, :, s, 0:hs, PAD:PAD + W] = pb[:, :, r0:r0 + hs, :]
    vplanes = np.stack([maskf, dcur, variance[:, 2], variance[:, 3],
                        pred_log[:, 1]], 1)
    vT = np.ascontiguousarray(vplanes.transpose(0, 1, 3, 2)).astype(NF16)
    vcol = np.zeros((nb, 5, 2, 128, FV), NF16)
    for c, (c0, cw) in enumerate(VCHUNKS):
        for s in range(NCS):
            bw = min(128, cw - s * 128)
            w0 = c0 + s * 128
            vcol[:, :, c, 0:bw, PAD + s * VSEG:PAD + s * VSEG + H] = \
                vT[:, :, w0:w0 + bw, :]
    return hrow, vcol


def _unpack(dout):
    """dout [3, 128, W] bf16 -> [H, W] f32."""
    return np.concatenate(
        [np.asarray(dout[s][0:hs], np.float32)
         for s, (r0, hs) in enumerate(RSEGS)], axis=0)


_NC = None


def _get_nc():
    global _NC
    if _NC is None:
        _NC = build_program()
    return _NC


def kernel(pred_log, mask, variance, depthin, lam, times):
    pred_log = np.asarray(pred_log, np.float32)
    mask = np.asarray(mask, np.int32)
    variance = np.asarray(variance, np.float32)
    depthin = np.asarray(depthin, np.float32)
    lam = np.asarray(lam, np.float32).reshape(1)
    t = int(np.asarray(times))

    if t <= 0:
        return depthin.copy()
    nc = _get_nc()
    maskf = mask[:, 0].astype(np.float32)
    dorig = depthin[:, 0]
    dcur = dorig
    for _ in range(t):
        hrow, vcol = _pack_inputs(pred_log, maskf, variance, dorig, dcur)
        in_maps = [{"hrow": hrow[b], "vcol": vcol[b], "lam": lam}
                   for b in range(B)]
        res = run_bass_kernel_spmd(nc, in_maps, list(range(B)))
        dcur = np.stack([_unpack(res.results[i]["dout"]) for i in range(B)])
    return dcur[:, None].astype(np.float32)
